# revision 48
# baseline (speedup 1.0000x reference)
"""Compact Bilinear Pooling (count-sketch + circular conv + spatial sum-pool)
as a Trainium2 Bass/Tile kernel, SPMD over 8 NeuronCores.

Math: with sk_i = flat @ S_i (flat: [B*P, C]), the reference computes
    out[b] = sum_{p in sample b} ifft( fft(sk1_p) * fft(sk2_p) ).real
Fold the sketch matrices into the DFT (Phi_i = fft(S_i, axis=1), half
spectrum k = 0..D/2 suffices since sk are real):
    G_m[p,k]  = x_p^T Phi_m[:,k]          (m: g1re g1im g2re g2im)
    Shat[b,k] = sum_{p in b} (G1 G2)[p,k]
    out[b,d]  = sum_k wk*(Re Shat * cos(2 pi k d/D) - Im Shat * sin)/D

v2 design (freqs on partitions, positions on the free dim):
  - 8-way FREQUENCY sharding: core j owns k in [512j, 512j+512) of the
    4096-padded half spectrum; host sums per-core partials.
  - Stage 1 on PE in fp8e4m3 DoubleRow (2 contraction chunks / instr at
    0.5 cyc/row): 3-term split x*Phi ~= x8@p8 + xlo@p8 + x8@plo keeps
    bf16-level accuracy at 3/4 the bf16 matmul cost (lo terms ride the
    e4m3 subnormal range).
    Per unit (kt freq-tile, sample): out g[128k, 4m, 196pos] psum.
  - ACT copies g psum->sbuf bf16; DVE forms the 4 cross-products with
    three strided muls; Pool (gpsimd) combines Re=RR-II, Im=RI+IR; DVE
    tensor_reduce sums positions per sample -> Shat[128k, 2, 16b].
    The DVE stream is software-pipelined by one group so the in-order
    engine never waits on its own group's Pool combine.
  - Stage 3 (inverse half-DFT, d and D-d folded): A = ccRe^T Sre,
    B = ccIm^T Sim accumulate in 2 psum banks across all kt,
    interleaved with stage 1 in 8-sample half-batches so only the last
    half-batch sits in the drain tail; host: out[d]=A+B, out[D-d]=A-B.
"""

import numpy as np
import ml_dtypes

import concourse.bacc as bacc
import concourse.mybir as mybir
import concourse.tile as tile
from concourse.bass_utils import run_bass_kernel_spmd

# problem dims (hardcoded per spec)
B, C, H, W, D = 16, 512, 14, 14, 8000
P = H * W             # 196 positions per sample
BP = B * P            # 3136
KH = D // 2 + 1       # 4001 half-spectrum frequencies
KPAD = 4096           # padded to 8*512
NCORES = 8
KSL = KPAD // NCORES  # 512 freqs per core
NKT = KSL // 128      # 4 freq tiles per core
NCC = C // 128        # 4 contraction chunks (channels)
DH = 4001             # folded output dim (d and D-d share tables)
NDT = 32              # 32 d-tiles of 128 (4096 padded)

F32 = mybir.dt.float32
BF16 = mybir.dt.bfloat16
FP8 = mybir.dt.float8e4
DR = mybir.MatmulPerfMode.DoubleRow

E4 = ml_dtypes.float8_e4m3


def build_nc():
    nc = bacc.Bacc("TRN2", target_bir_lowering=False, debug=False)
    # x_d[p, v, cc, n]: channel cc*128+p, position n; v in (x8, xlo)
    x_d = nc.dram_tensor("x", [128, 2, NCC, BP], FP8, kind="ExternalInput")
    # phi_d[p, v, kt, q, i, m, f]: PhiV_m[(2q+i)*128+p, 512j+128kt+f]
    phi_d = nc.dram_tensor("phi", [128, 2, NKT, 2, 2, 4, 128], FP8,
                           kind="ExternalInput")
    # cc_d[t, kt, p, dt*128+f]: t0 = wk*cos, t1 = -wk*sin
    cc_d = nc.dram_tensor("cc", [2, NKT, 128, NDT * 128], BF16,
                          kind="ExternalInput")
    out_d = nc.dram_tensor("out", [128, 2, B, NDT], BF16, kind="ExternalOutput")

    with tile.TileContext(nc) as tc:
        with (
            tc.tile_pool(name="xin", bufs=1) as x_pool,
            tc.tile_pool(name="phi", bufs=1) as phi_pool,
            tc.tile_pool(name="ccs", bufs=1) as cc_pool,
            tc.tile_pool(name="sbg", bufs=4) as sb_pool,
            tc.tile_pool(name="prd", bufs=3) as prod_pool,
            tc.tile_pool(name="com", bufs=3) as c_pool,
            tc.tile_pool(name="sht", bufs=4) as shat_pool,
            tc.tile_pool(name="stg", bufs=1) as st_pool,
            tc.tile_pool(name="gps", bufs=3, space="PSUM") as g_psum,
            tc.tile_pool(name="abs", bufs=1, space="PSUM") as ab_psum,
        ):
            # ---- PE warmup: ramp the clock through its p-states on dummy
            # matmuls while the first input DMAs are in flight
            warm = sb_pool.tile([128, 512], BF16, tag="warm")
            nc.vector.memset(warm[:], 0.0)
            wps = g_psum.tile([128, 4, 256], F32, tag="g", name="warmps")
            for w in range(10):
                nc.tensor.matmul(wps[:, 0:2].rearrange("p a b -> p (a b)"),
                                 lhsT=warm[:, 0:128], rhs=warm[:],
                                 start=True, stop=True, skip_group_check=True)

            # ---- inputs, ordered so unit (kt0, s0) can start ASAP and the
            # x stream stays ahead of kt0's sample consumption
            xt = x_pool.tile([128, 2, NCC, BP], FP8, tag="x")
            pt = phi_pool.tile([128, 2, NKT, 2, 2, 4, 128], FP8, tag="phi")
            nc.sync.dma_start(xt[:, :, :, 0:196], x_d.ap()[:, :, :, 0:196])
            nc.sync.dma_start(pt[:, :, 0], phi_d.ap()[:, :, 0])
            for c0, c1 in ((196, 392), (392, 1176), (1176, 2352),
                           (2352, BP)):
                nc.sync.dma_start(xt[:, :, :, c0:c1], x_d.ap()[:, :, :, c0:c1])
            nc.sync.dma_start(pt[:, :, 1:], phi_d.ap()[:, :, 1:])
            cct = {}
            for kt in range(NKT):
                for t in range(2):
                    ct = cc_pool.tile([128, NDT * 128], BF16, tag=f"cc{t}{kt}",
                                      name=f"cc{t}{kt}")
                    nc.sync.dma_start(ct[:], cc_d.ap()[t, kt])
                    cct[(t, kt)] = ct

            # ---- A/B accumulators (1 psum bank each, live whole kernel;
            # b-major so each 8-sample half is one contiguous 1KB region)
            apsum = ab_psum.tile([128, NDT, B], F32, tag="A")
            bpsum = ab_psum.tile([128, NDT, B], F32, tag="Bm")
            # explicit zero instead of matmul start=True: the b-major
            # strided writes only partially cover the bank's zero region,
            # which CoreSim's pending-zero model cannot express
            nc.vector.memset(apsum[:], 0.0)
            nc.vector.memset(bpsum[:], 0.0)

            def stage3(kt, h, final=False):
                # A[dt] += ccRe[kt]^T Sre[kt];  B[dt] += ccIm[kt]^T Sim[kt]
                # for the 8-sample half h (banks pre-zeroed by memset).
                # On the final piece, drain A to HBM while PE runs B.
                b0 = 8 * h
                for t, ps in ((0, apsum), (1, bpsum)):
                    for dt in range(NDT):
                        nc.tensor.matmul(
                            ps[:, dt, b0:b0 + 8],
                            lhsT=cct[(t, kt)][:, dt * 128:(dt + 1) * 128],
                            rhs=shat[kt][:, t, b0:b0 + 8],
                            start=False,
                            # kt3 runs h1 before h0, so h0 closes the group
                            stop=(final and dt == NDT - 1),
                            skip_group_check=True,
                        )
                    if final:
                        drain_one(t, h)

            shat = [shat_pool.tile([128, 2, B], BF16, tag=f"shat{kt}",
                                   name=f"shat{kt}")
                    for kt in range(NKT)]
            shatf = [shat_pool.tile([128, 2, B], F32, tag=f"shatf{kt}",
                                    name=f"shatf{kt}")
                     for kt in range(NKT)]

            # ---- main loop: kt-major, 16 samples each. The DVE stream is
            # software-pipelined by one group: reduce(i) is emitted after
            # muls(i+1) so the in-order DVE never stalls on Pool(i).
            pending = []  # (kt, s, cm) awaiting the position-reduce

            def flush_reduce():
                while pending:
                    fkt, fs, fw, fcm = pending.pop(0)
                    nc.vector.tensor_reduce(
                        shatf[fkt][:, :, fs - fw + 1:fs + 1],
                        fcm[:, :, 0:fw, 0:P],
                        axis=mybir.AxisListType.X, op=mybir.AluOpType.add)
                    if fs == 7 or fs == 15:
                        b0 = 0 if fs == 7 else 8
                        nc.scalar.copy(shat[fkt][:, :, b0:b0 + 8],
                                       shatf[fkt][:, :, b0:b0 + 8])

            stage = st_pool.tile([128, 2, B, NDT], BF16, tag="stage")

            def drain_one(t, h):
                # A or B psum -> bf16 -> HBM for sample half h (columns are
                # final once stage3(kt3, h) has run for that bank; the rest
                # of the banks may still be accumulating).  dt-major psum is
                # transposed into the b-major stage so the out DMA moves
                # contiguous 512B-per-sample runs.
                b0 = 8 * h
                ps = apsum if t == 0 else bpsum
                nc.scalar.copy(stage[:, t, b0:b0 + 8],
                               ps[:, :, b0:b0 + 8]
                               .rearrange("p d b -> p b d"))
                # alternate DMA queues so the two final issue trains
                # (seq + hwdge + dge latency) overlap
                eng = nc.sync if t == 0 else nc.scalar
                eng.dma_start(out_d.ap()[:, t, b0:b0 + 8],
                              stage[:, t, b0:b0 + 8])

            # kt3 processes its h1 samples first so that only h0's final
            # groups sit in the end-of-kernel drain tail
            for kt in range(NKT):
                order = (list(range(8, 16)) + list(range(8))
                         if kt == NKT - 1 else list(range(B)))
                for u in range(B):
                    s = order[u]
                    g = g_psum.tile([128, 4, 256], F32, tag="g",
                                    name=f"g{kt}_{s}")
                    n0 = s * P
                    for m in range(4):
                        for xv, pv in ((0, 0), (1, 0), (0, 1)):
                            for q in range(2):
                                nc.tensor.matmul(
                                    g[:, m, 0:P],
                                    lhsT=pt[:, pv, kt, q, :, m, :],
                                    rhs=xt[:, xv, 2 * q:2 * q + 2, n0:n0 + P],
                                    start=(m % 2 == 0 and xv == 0 and pv == 0
                                           and q == 0),
                                    stop=(m % 2 == 1 and pv == 1 and q == 1),
                                    perf_mode=DR,
                                    skip_group_check=True,
                                )
                    # ACT: psum f32 -> sbuf bf16; 2 samples per vector group
                    fast = (kt == NKT - 1 and u == B - 1)
                    gw = 2
                    if u % 2 == 0:
                        sbt = sb_pool.tile([128, 4, gw, P], BF16,
                                           tag=f"sb{gw}", name=f"sb{kt}_{s}")
                    nc.scalar.copy(sbt[:, :, u % 2, :], g[:, :, 0:P])
                    if u % 2 == 1:
                        # DVE computes (RR, II) and the Re combine locally;
                        # Pool computes the (RI, IR) pair in one negative-
                        # stride mul so only the Im combine crosses engines.
                        pr = prod_pool.tile([128, 4, gw, P], BF16,
                                            tag=f"pr{gw}", name=f"pr{kt}_{s}")
                        nc.vector.tensor_mul(pr[:, 0:2], sbt[:, 0:2],
                                             sbt[:, 2:4])
                        nc.gpsimd.tensor_mul(pr[:, 2:4], sbt[:, 0:2],
                                             sbt[:, 3:1:-1])
                        # (cm free-dim padded to 256 so the AP optimizer
                        # cannot merge dims — the reduce window must be 196)
                        cm = c_pool.tile([128, 2, gw, 256], BF16,
                                         tag=f"cm{gw}", name=f"cm{kt}_{s}")
                        nc.vector.tensor_sub(cm[:, 0, :, 0:P], pr[:, 0],
                                             pr[:, 1])
                        flush_reduce()
                        nc.vector.tensor_add(cm[:, 1, :, 0:P], pr[:, 2],
                                             pr[:, 3])
                        pending.append((kt, s, gw, cm))
                        if fast:
                            flush_reduce()
                    # stage-3 half-batches: a half's slot sits >= 5 groups
                    # (~10us of vector-pipe drain) after its last sample;
                    # kt0's slots additionally wait out the cc DMA arrival
                    if kt >= 1 and u == (4 if kt == 1 else 2):
                        stage3(kt - 1, 0)
                    if kt >= 1 and u == 10:
                        stage3(kt - 1, 1)
                    if kt == NKT - 1 and u == 13:
                        stage3(kt, 1)
                        drain_one(0, 1)
                        drain_one(1, 1)
            stage3(NKT - 1, 0, final=True)

    nc.compile()
    return nc


def make_constants(S1, S2):
    """Host-side constant prep from the sketch matrices (per-core slices)."""
    S1 = np.asarray(S1, np.float64)
    S2 = np.asarray(S2, np.float64)
    Phi = np.zeros((4, C, KPAD), np.float32)
    for i, S in enumerate((S1, S2)):
        F = np.fft.fft(S, axis=1)[:, :KH]
        Phi[2 * i, :, :KH] = F.real.astype(np.float32)
        Phi[2 * i + 1, :, :KH] = F.imag.astype(np.float32)

    # fp8 hi/lo split of Phi (lo rides e4m3 subnormals)
    p8 = Phi.astype(E4)
    plo = (Phi - p8.astype(np.float32)).astype(E4)

    # phi layout [128, v, kt, q, i, m, f]; v = (p8, plo)
    phis = []
    stack = np.stack([p8, plo], 0)  # [v, m, c, k]
    arr = stack.reshape(2, 4, 2, 2, 128, NCORES, NKT, 128)  # v m q i p j kt f
    for j in range(NCORES):
        a = arr[:, :, :, :, :, j]  # [v, m, q, i, p, kt, f]
        a = np.transpose(a, (4, 0, 5, 2, 3, 1, 6))  # p v kt q i m f
        phis.append(np.ascontiguousarray(a))

    # inverse half-DFT tables, d/D-d folded
    k = np.arange(KPAD, dtype=np.float64)
    wk = np.where((k == 0) | (k == D // 2), 1.0, 2.0) / D
    wk[KH:] = 0.0
    ang = 2.0 * np.pi * np.outer(k, np.arange(DH, dtype=np.float64)) / D
    Cst = np.zeros((2, KPAD, NDT * 128), np.float32)
    Cst[0, :, :DH] = wk[:, None] * np.cos(ang)
    Cst[1, :, :DH] = -wk[:, None] * np.sin(ang)
    Cst = Cst.astype(ml_dtypes.bfloat16)
    ccs = [np.ascontiguousarray(
        Cst.reshape(2, NCORES, NKT, 128, NDT * 128)[:, j]) for j in range(NCORES)]
    return phis, ccs


def prep_x(x):
    """[B, C, H, W] -> [128, 2, cc, BP] fp8 (x8, xlo)."""
    xr = np.ascontiguousarray(
        np.asarray(x, np.float32).reshape(B, C, P).transpose(1, 0, 2)
    ).reshape(C, BP)
    x8 = xr.astype(E4)
    xlo = (xr - x8.astype(np.float32)).astype(E4)
    out = np.empty((128, 2, NCC, BP), E4)
    for v, t in enumerate((x8, xlo)):
        out[:, v] = t.reshape(NCC, 128, BP).transpose(1, 0, 2)
    return out


def unshard(parts):
    """Sum per-core [128, 2, NDT, B] bf16 partials -> [B, D] f32."""
    acc = np.zeros((2, NDT * 128, B), np.float32)
    for r in parts:
        a = np.asarray(r, np.float32)  # [128, 2, B, NDT]
        acc += a.transpose(1, 3, 0, 2).reshape(2, NDT * 128, B)
    A, Bm = acc[0], acc[1]
    out = np.zeros((D, B), np.float32)
    out[:KH] = A[:KH] + Bm[:KH]
    out[KH:] = (A[1:4000] - Bm[1:4000])[::-1]
    return np.ascontiguousarray(out.T)


_CACHE = {}


def kernel(x, S1, S2):
    x = np.asarray(x)
    if "k" not in _CACHE:
        phis, ccs = make_constants(np.asarray(S1), np.asarray(S2))
        _CACHE["k"] = (build_nc(), phis, ccs)
    nc, phis, ccs = _CACHE["k"]

    xp = prep_x(x)
    in_maps = [{"x": xp, "phi": phis[j], "cc": ccs[j]} for j in range(NCORES)]
    res = run_bass_kernel_spmd(nc, in_maps, list(range(NCORES)))
    return unshard([r["out"] for r in res.results]).astype(x.dtype)


# revision 49
# speedup vs baseline: 1.0059x; 1.0059x over previous
"""Compact Bilinear Pooling (count-sketch + circular conv + spatial sum-pool)
as a Trainium2 Bass/Tile kernel, SPMD over 8 NeuronCores.

Math: with sk_i = flat @ S_i (flat: [B*P, C]), the reference computes
    out[b] = sum_{p in sample b} ifft( fft(sk1_p) * fft(sk2_p) ).real
Fold the sketch matrices into the DFT (Phi_i = fft(S_i, axis=1), half
spectrum k = 0..D/2 suffices since sk are real):
    G_m[p,k]  = x_p^T Phi_m[:,k]          (m: g1re g1im g2re g2im)
    Shat[b,k] = sum_{p in b} (G1 G2)[p,k]
    out[b,d]  = sum_k wk*(Re Shat * cos(2 pi k d/D) - Im Shat * sin)/D

v2 design (freqs on partitions, positions on the free dim):
  - 8-way FREQUENCY sharding: core j owns k in [512j, 512j+512) of the
    4096-padded half spectrum; host sums per-core partials.
  - Stage 1 on PE in fp8e4m3 DoubleRow (2 contraction chunks / instr at
    0.5 cyc/row): 3-term split x*Phi ~= x8@p8 + xlo@p8 + x8@plo keeps
    bf16-level accuracy at 3/4 the bf16 matmul cost (lo terms ride the
    e4m3 subnormal range).
    Per unit (kt freq-tile, sample): out g[128k, 4m, 196pos] psum.
  - ACT copies g psum->sbuf bf16; DVE forms the 4 cross-products with
    three strided muls; Pool (gpsimd) combines Re=RR-II, Im=RI+IR; DVE
    tensor_reduce sums positions per sample -> Shat[128k, 2, 16b].
    The DVE stream is software-pipelined by one group so the in-order
    engine never waits on its own group's Pool combine.
  - Stage 3 (inverse half-DFT, d and D-d folded): A = ccRe^T Sre,
    B = ccIm^T Sim accumulate in 2 psum banks across all kt,
    interleaved with stage 1 in 8-sample half-batches so only the last
    half-batch sits in the drain tail; host: out[d]=A+B, out[D-d]=A-B.
"""

import numpy as np
import ml_dtypes

import concourse.bacc as bacc
import concourse.mybir as mybir
import concourse.tile as tile
from concourse.bass_utils import run_bass_kernel_spmd

# problem dims (hardcoded per spec)
B, C, H, W, D = 16, 512, 14, 14, 8000
P = H * W             # 196 positions per sample
BP = B * P            # 3136
KH = D // 2 + 1       # 4001 half-spectrum frequencies
KPAD = 4096           # padded to 8*512
NCORES = 8
KSL = KPAD // NCORES  # 512 freqs per core
NKT = KSL // 128      # 4 freq tiles per core
NCC = C // 128        # 4 contraction chunks (channels)
DH = 4001             # folded output dim (d and D-d share tables)
NDT = 32              # 32 d-tiles of 128 (4096 padded)

F32 = mybir.dt.float32
BF16 = mybir.dt.bfloat16
FP8 = mybir.dt.float8e4
DR = mybir.MatmulPerfMode.DoubleRow

E4 = ml_dtypes.float8_e4m3


def build_nc():
    nc = bacc.Bacc("TRN2", target_bir_lowering=False, debug=False)
    # x_d[p, v, cc, n]: channel cc*128+p, position n; v in (x8, xlo)
    x_d = nc.dram_tensor("x", [128, 2, NCC, BP], FP8, kind="ExternalInput")
    # phi_d[p, v, kt, q, i, m, f]: PhiV_m[(2q+i)*128+p, 512j+128kt+f]
    phi_d = nc.dram_tensor("phi", [128, 2, NKT, 2, 2, 4, 128], FP8,
                           kind="ExternalInput")
    # cc_d[t, kt, p, dt*128+f]: t0 = wk*cos, t1 = -wk*sin
    cc_d = nc.dram_tensor("cc", [2, NKT, 128, NDT * 128], BF16,
                          kind="ExternalInput")
    out_d = nc.dram_tensor("out", [128, 2, B, NDT], BF16, kind="ExternalOutput")

    with tile.TileContext(nc) as tc:
        with (
            tc.tile_pool(name="xin", bufs=1) as x_pool,
            tc.tile_pool(name="phi", bufs=1) as phi_pool,
            tc.tile_pool(name="ccs", bufs=1) as cc_pool,
            tc.tile_pool(name="sbg", bufs=4) as sb_pool,
            tc.tile_pool(name="prd", bufs=3) as prod_pool,
            tc.tile_pool(name="com", bufs=3) as c_pool,
            tc.tile_pool(name="sht", bufs=4) as shat_pool,
            tc.tile_pool(name="stg", bufs=1) as st_pool,
            tc.tile_pool(name="gps", bufs=3, space="PSUM") as g_psum,
            tc.tile_pool(name="abs", bufs=1, space="PSUM") as ab_psum,
        ):
            # ---- PE warmup: ramp the clock through its p-states on dummy
            # matmuls while the first input DMAs are in flight
            warm = sb_pool.tile([128, 512], BF16, tag="warm")
            nc.vector.memset(warm[:], 0.0)
            wps = g_psum.tile([128, 4, 256], F32, tag="g", name="warmps")
            for w in range(10):
                nc.tensor.matmul(wps[:, 0:2].rearrange("p a b -> p (a b)"),
                                 lhsT=warm[:, 0:128], rhs=warm[:],
                                 start=True, stop=True, skip_group_check=True)

            # ---- inputs, ordered so unit (kt0, s0) can start ASAP and the
            # x stream stays ahead of kt0's sample consumption
            xt = x_pool.tile([128, 2, NCC, BP], FP8, tag="x")
            pt = phi_pool.tile([128, 2, NKT, 2, 2, 4, 128], FP8, tag="phi")
            nc.sync.dma_start(xt[:, :, :, 0:196], x_d.ap()[:, :, :, 0:196])
            nc.sync.dma_start(pt[:, :, 0], phi_d.ap()[:, :, 0])
            for c0, c1 in ((196, 392), (392, 1176), (1176, 2352),
                           (2352, BP)):
                nc.sync.dma_start(xt[:, :, :, c0:c1], x_d.ap()[:, :, :, c0:c1])
            nc.sync.dma_start(pt[:, :, 1:], phi_d.ap()[:, :, 1:])
            cct = {}
            for kt in range(NKT):
                for t in range(2):
                    ct = cc_pool.tile([128, NDT * 128], BF16, tag=f"cc{t}{kt}",
                                      name=f"cc{t}{kt}")
                    nc.sync.dma_start(ct[:], cc_d.ap()[t, kt])
                    cct[(t, kt)] = ct

            # ---- A/B accumulators (1 psum bank each, live whole kernel;
            # b-major so each 8-sample half is one contiguous 1KB region)
            apsum = ab_psum.tile([128, NDT, B], F32, tag="A")
            bpsum = ab_psum.tile([128, NDT, B], F32, tag="Bm")
            # explicit zero instead of matmul start=True: the b-major
            # strided writes only partially cover the bank's zero region,
            # which CoreSim's pending-zero model cannot express
            nc.vector.memset(apsum[:], 0.0)
            nc.vector.memset(bpsum[:], 0.0)

            def stage3(kt, h, final=False):
                # A[dt] += ccRe[kt]^T Sre[kt];  B[dt] += ccIm[kt]^T Sim[kt]
                # for the 8-sample half h (banks pre-zeroed by memset).
                # On the final piece, drain A to HBM while PE runs B.
                b0 = 8 * h
                for t, ps in ((0, apsum), (1, bpsum)):
                    for dt in range(NDT):
                        nc.tensor.matmul(
                            ps[:, dt, b0:b0 + 8],
                            lhsT=cct[(t, kt)][:, dt * 128:(dt + 1) * 128],
                            rhs=shat[kt][:, t, b0:b0 + 8],
                            start=False,
                            # kt3 runs h1 before h0, so h0 closes the group
                            stop=(final and dt == NDT - 1),
                            skip_group_check=True,
                        )
                    if final:
                        drain_one(t, h)

            shat = [shat_pool.tile([128, 2, B], BF16, tag=f"shat{kt}",
                                   name=f"shat{kt}")
                    for kt in range(NKT)]
            shatf = [shat_pool.tile([128, 2, B], F32, tag=f"shatf{kt}",
                                    name=f"shatf{kt}")
                     for kt in range(NKT)]

            # ---- main loop: kt-major, 16 samples each. The DVE stream is
            # software-pipelined by one group: reduce(i) is emitted after
            # muls(i+1) so the in-order DVE never stalls on Pool(i).
            pending = []  # (kt, s, cm) awaiting the position-reduce

            def flush_reduce():
                while pending:
                    fkt, fs, fw, fcm = pending.pop(0)
                    nc.vector.tensor_reduce(
                        shatf[fkt][:, :, fs - fw + 1:fs + 1],
                        fcm[:, :, 0:fw, 0:P],
                        axis=mybir.AxisListType.X, op=mybir.AluOpType.add)
                    if fs == 7 or fs == 15:
                        b0 = 0 if fs == 7 else 8
                        nc.scalar.copy(shat[fkt][:, :, b0:b0 + 8],
                                       shatf[fkt][:, :, b0:b0 + 8])

            stage = st_pool.tile([128, 2, B, NDT], BF16, tag="stage")

            def drain_one(t, h):
                # A or B psum -> bf16 -> HBM for sample half h (columns are
                # final once stage3(kt3, h) has run for that bank; the rest
                # of the banks may still be accumulating).  dt-major psum is
                # transposed into the b-major stage so the out DMA moves
                # contiguous 512B-per-sample runs.
                b0 = 8 * h
                ps = apsum if t == 0 else bpsum
                nc.scalar.copy(stage[:, t, b0:b0 + 8],
                               ps[:, :, b0:b0 + 8]
                               .rearrange("p d b -> p b d"))
                # alternate DMA queues so the two final issue trains
                # (seq + hwdge + dge latency) overlap
                eng = nc.sync if t == 0 else nc.scalar
                eng.dma_start(out_d.ap()[:, t, b0:b0 + 8],
                              stage[:, t, b0:b0 + 8])

            # kt3 processes its h1 samples first so that only h0's final
            # groups sit in the end-of-kernel drain tail
            for kt in range(NKT):
                order = (list(range(8, 16)) + list(range(8))
                         if kt == NKT - 1 else list(range(B)))
                for u in range(B):
                    s = order[u]
                    g = g_psum.tile([128, 4, 256], F32, tag="g",
                                    name=f"g{kt}_{s}")
                    n0 = s * P
                    for m in range(4):
                        for xv, pv in ((0, 0), (1, 0), (0, 1)):
                            for q in range(2):
                                nc.tensor.matmul(
                                    g[:, m, 0:P],
                                    lhsT=pt[:, pv, kt, q, :, m, :],
                                    rhs=xt[:, xv, 2 * q:2 * q + 2, n0:n0 + P],
                                    start=(m % 2 == 0 and xv == 0 and pv == 0
                                           and q == 0),
                                    stop=(m % 2 == 1 and pv == 1 and q == 1),
                                    perf_mode=DR,
                                    skip_group_check=True,
                                )
                    # ACT: psum f32 -> sbuf bf16; 2 samples per vector group
                    fast = (kt == NKT - 1 and u == B - 1)
                    gw = 2
                    if u % 2 == 0:
                        sbt = sb_pool.tile([128, 4, gw, P], BF16,
                                           tag=f"sb{gw}", name=f"sb{kt}_{s}")
                    nc.scalar.copy(sbt[:, :, u % 2, :], g[:, :, 0:P])
                    if u % 2 == 1:
                        # DVE computes (RR, II) and the Re combine locally;
                        # Pool computes the (RI, IR) pair in one negative-
                        # stride mul so only the Im combine crosses engines.
                        pr = prod_pool.tile([128, 4, gw, P], BF16,
                                            tag=f"pr{gw}", name=f"pr{kt}_{s}")
                        nc.vector.tensor_mul(pr[:, 0:2], sbt[:, 0:2],
                                             sbt[:, 2:4])
                        nc.gpsimd.tensor_mul(pr[:, 2:4], sbt[:, 0:2],
                                             sbt[:, 3:1:-1])
                        # (cm free-dim padded to 256 so the AP optimizer
                        # cannot merge dims — the reduce window must be 196)
                        cm = c_pool.tile([128, 2, gw, 256], BF16,
                                         tag=f"cm{gw}", name=f"cm{kt}_{s}")
                        nc.vector.tensor_sub(cm[:, 0, :, 0:P], pr[:, 0],
                                             pr[:, 1])
                        flush_reduce()
                        nc.vector.tensor_add(cm[:, 1, :, 0:P], pr[:, 2],
                                             pr[:, 3])
                        pending.append((kt, s, gw, cm))
                        if fast:
                            flush_reduce()
                    # stage-3 half-batches: a half's slot sits >= 5 groups
                    # (~10us of vector-pipe drain) after its last sample;
                    # kt0's slots additionally wait out the cc DMA arrival
                    if kt >= 1 and u == (4 if kt == 1 else 2):
                        stage3(kt - 1, 0)
                    if kt >= 1 and u == 10:
                        stage3(kt - 1, 1)
            stage3(NKT - 1, 1)
            drain_one(0, 1)
            drain_one(1, 1)
            stage3(NKT - 1, 0, final=True)

    nc.compile()
    return nc


def make_constants(S1, S2):
    """Host-side constant prep from the sketch matrices (per-core slices)."""
    S1 = np.asarray(S1, np.float64)
    S2 = np.asarray(S2, np.float64)
    Phi = np.zeros((4, C, KPAD), np.float32)
    for i, S in enumerate((S1, S2)):
        F = np.fft.fft(S, axis=1)[:, :KH]
        Phi[2 * i, :, :KH] = F.real.astype(np.float32)
        Phi[2 * i + 1, :, :KH] = F.imag.astype(np.float32)

    # fp8 hi/lo split of Phi (lo rides e4m3 subnormals)
    p8 = Phi.astype(E4)
    plo = (Phi - p8.astype(np.float32)).astype(E4)

    # phi layout [128, v, kt, q, i, m, f]; v = (p8, plo)
    phis = []
    stack = np.stack([p8, plo], 0)  # [v, m, c, k]
    arr = stack.reshape(2, 4, 2, 2, 128, NCORES, NKT, 128)  # v m q i p j kt f
    for j in range(NCORES):
        a = arr[:, :, :, :, :, j]  # [v, m, q, i, p, kt, f]
        a = np.transpose(a, (4, 0, 5, 2, 3, 1, 6))  # p v kt q i m f
        phis.append(np.ascontiguousarray(a))

    # inverse half-DFT tables, d/D-d folded
    k = np.arange(KPAD, dtype=np.float64)
    wk = np.where((k == 0) | (k == D // 2), 1.0, 2.0) / D
    wk[KH:] = 0.0
    ang = 2.0 * np.pi * np.outer(k, np.arange(DH, dtype=np.float64)) / D
    Cst = np.zeros((2, KPAD, NDT * 128), np.float32)
    Cst[0, :, :DH] = wk[:, None] * np.cos(ang)
    Cst[1, :, :DH] = -wk[:, None] * np.sin(ang)
    Cst = Cst.astype(ml_dtypes.bfloat16)
    ccs = [np.ascontiguousarray(
        Cst.reshape(2, NCORES, NKT, 128, NDT * 128)[:, j]) for j in range(NCORES)]
    return phis, ccs


def prep_x(x):
    """[B, C, H, W] -> [128, 2, cc, BP] fp8 (x8, xlo)."""
    xr = np.ascontiguousarray(
        np.asarray(x, np.float32).reshape(B, C, P).transpose(1, 0, 2)
    ).reshape(C, BP)
    x8 = xr.astype(E4)
    xlo = (xr - x8.astype(np.float32)).astype(E4)
    out = np.empty((128, 2, NCC, BP), E4)
    for v, t in enumerate((x8, xlo)):
        out[:, v] = t.reshape(NCC, 128, BP).transpose(1, 0, 2)
    return out


def unshard(parts):
    """Sum per-core [128, 2, NDT, B] bf16 partials -> [B, D] f32."""
    acc = np.zeros((2, NDT * 128, B), np.float32)
    for r in parts:
        a = np.asarray(r, np.float32)  # [128, 2, B, NDT]
        acc += a.transpose(1, 3, 0, 2).reshape(2, NDT * 128, B)
    A, Bm = acc[0], acc[1]
    out = np.zeros((D, B), np.float32)
    out[:KH] = A[:KH] + Bm[:KH]
    out[KH:] = (A[1:4000] - Bm[1:4000])[::-1]
    return np.ascontiguousarray(out.T)


_CACHE = {}


def kernel(x, S1, S2):
    x = np.asarray(x)
    if "k" not in _CACHE:
        phis, ccs = make_constants(np.asarray(S1), np.asarray(S2))
        _CACHE["k"] = (build_nc(), phis, ccs)
    nc, phis, ccs = _CACHE["k"]

    xp = prep_x(x)
    in_maps = [{"x": xp, "phi": phis[j], "cc": ccs[j]} for j in range(NCORES)]
    res = run_bass_kernel_spmd(nc, in_maps, list(range(NCORES)))
    return unshard([r["out"] for r in res.results]).astype(x.dtype)


# revision 51
# speedup vs baseline: 1.0088x; 1.0028x over previous
"""Compact Bilinear Pooling (count-sketch + circular conv + spatial sum-pool)
as a Trainium2 Bass/Tile kernel, SPMD over 8 NeuronCores.

Math: with sk_i = flat @ S_i (flat: [B*P, C]), the reference computes
    out[b] = sum_{p in sample b} ifft( fft(sk1_p) * fft(sk2_p) ).real
Fold the sketch matrices into the DFT (Phi_i = fft(S_i, axis=1), half
spectrum k = 0..D/2 suffices since sk are real):
    G_m[p,k]  = x_p^T Phi_m[:,k]          (m: g1re g1im g2re g2im)
    Shat[b,k] = sum_{p in b} (G1 G2)[p,k]
    out[b,d]  = sum_k wk*(Re Shat * cos(2 pi k d/D) - Im Shat * sin)/D

v2 design (freqs on partitions, positions on the free dim):
  - 8-way FREQUENCY sharding: core j owns k in [512j, 512j+512) of the
    4096-padded half spectrum; host sums per-core partials.
  - Stage 1 on PE in fp8e4m3 DoubleRow (2 contraction chunks / instr at
    0.5 cyc/row): 3-term split x*Phi ~= x8@p8 + xlo@p8 + x8@plo keeps
    bf16-level accuracy at 3/4 the bf16 matmul cost (lo terms ride the
    e4m3 subnormal range).
    Per unit (kt freq-tile, sample): out g[128k, 4m, 196pos] psum.
  - ACT copies g psum->sbuf bf16; DVE forms the 4 cross-products with
    three strided muls; Pool (gpsimd) combines Re=RR-II, Im=RI+IR; DVE
    tensor_reduce sums positions per sample -> Shat[128k, 2, 16b].
    The DVE stream is software-pipelined by one group so the in-order
    engine never waits on its own group's Pool combine.
  - Stage 3 (inverse half-DFT, d and D-d folded): A = ccRe^T Sre,
    B = ccIm^T Sim accumulate in 2 psum banks across all kt,
    interleaved with stage 1 in 8-sample half-batches so only the last
    half-batch sits in the drain tail; host: out[d]=A+B, out[D-d]=A-B.
"""

import numpy as np
import ml_dtypes

import concourse.bacc as bacc
import concourse.mybir as mybir
import concourse.tile as tile
from concourse.bass_utils import run_bass_kernel_spmd

# problem dims (hardcoded per spec)
B, C, H, W, D = 16, 512, 14, 14, 8000
P = H * W             # 196 positions per sample
BP = B * P            # 3136
KH = D // 2 + 1       # 4001 half-spectrum frequencies
KPAD = 4096           # padded to 8*512
NCORES = 8
KSL = KPAD // NCORES  # 512 freqs per core
NKT = KSL // 128      # 4 freq tiles per core
NCC = C // 128        # 4 contraction chunks (channels)
DH = 4001             # folded output dim (d and D-d share tables)
NDT = 32              # 32 d-tiles of 128 (4096 padded)

F32 = mybir.dt.float32
BF16 = mybir.dt.bfloat16
FP8 = mybir.dt.float8e4
DR = mybir.MatmulPerfMode.DoubleRow

E4 = ml_dtypes.float8_e4m3


def build_nc():
    nc = bacc.Bacc("TRN2", target_bir_lowering=False, debug=False)
    # x_d[p, v, cc, n]: channel cc*128+p, position n; v in (x8, xlo)
    x_d = nc.dram_tensor("x", [128, 2, NCC, BP], FP8, kind="ExternalInput")
    # phi_d[p, v, kt, q, i, m, f]: PhiV_m[(2q+i)*128+p, 512j+128kt+f]
    phi_d = nc.dram_tensor("phi", [128, 2, NKT, 2, 2, 4, 128], FP8,
                           kind="ExternalInput")
    # cc_d[t, kt, p, dt*128+f]: t0 = wk*cos, t1 = -wk*sin
    cc_d = nc.dram_tensor("cc", [2, NKT, 128, NDT * 128], BF16,
                          kind="ExternalInput")
    out_d = nc.dram_tensor("out", [128, 2, B, NDT], BF16, kind="ExternalOutput")

    with tile.TileContext(nc) as tc:
        with (
            tc.tile_pool(name="xin", bufs=1) as x_pool,
            tc.tile_pool(name="phi", bufs=1) as phi_pool,
            tc.tile_pool(name="ccs", bufs=1) as cc_pool,
            tc.tile_pool(name="sbg", bufs=4) as sb_pool,
            tc.tile_pool(name="prd", bufs=3) as prod_pool,
            tc.tile_pool(name="com", bufs=3) as c_pool,
            tc.tile_pool(name="sht", bufs=4) as shat_pool,
            tc.tile_pool(name="stg", bufs=1) as st_pool,
            tc.tile_pool(name="gps", bufs=3, space="PSUM") as g_psum,
            tc.tile_pool(name="abs", bufs=1, space="PSUM") as ab_psum,
        ):
            # ---- PE warmup: ramp the clock through its p-states on dummy
            # matmuls while the first input DMAs are in flight
            warm = sb_pool.tile([128, 512], BF16, tag="warm")
            nc.vector.memset(warm[:], 0.0)
            wps = g_psum.tile([128, 4, 256], F32, tag="g", name="warmps")
            for w in range(10):
                nc.tensor.matmul(wps[:, 0:2].rearrange("p a b -> p (a b)"),
                                 lhsT=warm[:, 0:128], rhs=warm[:],
                                 start=True, stop=True, skip_group_check=True)

            # ---- inputs, ordered so unit (kt0, s0) can start ASAP and the
            # x stream stays ahead of kt0's sample consumption
            xt = x_pool.tile([128, 2, NCC, BP], FP8, tag="x")
            pt = phi_pool.tile([128, 2, NKT, 2, 2, 4, 128], FP8, tag="phi")
            nc.sync.dma_start(xt[:, :, :, 0:196], x_d.ap()[:, :, :, 0:196])
            nc.sync.dma_start(pt[:, :, 0], phi_d.ap()[:, :, 0])
            for c0, c1 in ((196, 392), (392, 1176), (1176, 2352),
                           (2352, BP)):
                nc.sync.dma_start(xt[:, :, :, c0:c1], x_d.ap()[:, :, :, c0:c1])
            nc.sync.dma_start(pt[:, :, 1:], phi_d.ap()[:, :, 1:])
            cct = {}
            for kt in range(NKT):
                for t in range(2):
                    ct = cc_pool.tile([128, NDT * 128], BF16, tag=f"cc{t}{kt}",
                                      name=f"cc{t}{kt}")
                    nc.sync.dma_start(ct[:], cc_d.ap()[t, kt])
                    cct[(t, kt)] = ct

            # ---- A/B accumulators (1 psum bank each, live whole kernel;
            # b-major so each 8-sample half is one contiguous 1KB region)
            apsum = ab_psum.tile([128, NDT, B], F32, tag="A")
            bpsum = ab_psum.tile([128, NDT, B], F32, tag="Bm")
            # explicit zero instead of matmul start=True: the b-major
            # strided writes only partially cover the bank's zero region,
            # which CoreSim's pending-zero model cannot express
            nc.vector.memset(apsum[:], 0.0)
            nc.vector.memset(bpsum[:], 0.0)

            def stage3(kt, h, final=False):
                # A[dt] += ccRe[kt]^T Sre[kt];  B[dt] += ccIm[kt]^T Sim[kt]
                # for the 8-sample half h (banks pre-zeroed by memset).
                # On the final piece, drain A to HBM while PE runs B.
                b0 = 8 * h
                for t, ps in ((0, apsum), (1, bpsum)):
                    for dt in range(NDT):
                        nc.tensor.matmul(
                            ps[:, dt, b0:b0 + 8],
                            lhsT=cct[(t, kt)][:, dt * 128:(dt + 1) * 128],
                            rhs=shat[kt][:, t, b0:b0 + 8],
                            start=False,
                            # kt3 runs h1 before h0, so h0 closes the group
                            stop=(final and dt == NDT - 1),
                            skip_group_check=True,
                        )
                    if final:
                        copy_one(t, h)
                if final:
                    # one fused DMA for both banks: a second issue train
                    # (seq + hwdge + dge latency) would outcost the bytes
                    nc.sync.dma_start(out_d.ap()[:, :, b0:b0 + 8],
                                      stage[:, :, b0:b0 + 8])

            shat = [shat_pool.tile([128, 2, B], BF16, tag=f"shat{kt}",
                                   name=f"shat{kt}")
                    for kt in range(NKT)]
            shatf = [shat_pool.tile([128, 2, B], F32, tag=f"shatf{kt}",
                                    name=f"shatf{kt}")
                     for kt in range(NKT)]

            # ---- main loop: kt-major, 16 samples each. The DVE stream is
            # software-pipelined by one group: reduce(i) is emitted after
            # muls(i+1) so the in-order DVE never stalls on Pool(i).
            pending = []  # (kt, s, cm) awaiting the position-reduce

            def flush_reduce():
                while pending:
                    fkt, fs, fw, fcm = pending.pop(0)
                    nc.vector.tensor_reduce(
                        shatf[fkt][:, :, fs - fw + 1:fs + 1],
                        fcm[:, :, 0:fw, 0:P],
                        axis=mybir.AxisListType.X, op=mybir.AluOpType.add)
                    if fs == 7 or fs == 15:
                        b0 = 0 if fs == 7 else 8
                        nc.scalar.copy(shat[fkt][:, :, b0:b0 + 8],
                                       shatf[fkt][:, :, b0:b0 + 8])

            stage = st_pool.tile([128, 2, B, NDT], BF16, tag="stage")

            def copy_one(t, h):
                # A or B psum -> bf16 stage for sample half h (columns are
                # final once stage3(kt3, h) has run for that bank; the rest
                # of the banks may still be accumulating).  dt-major psum is
                # transposed into the b-major stage so the out DMA moves
                # contiguous 512B-per-sample runs.
                b0 = 8 * h
                ps = apsum if t == 0 else bpsum
                nc.scalar.copy(stage[:, t, b0:b0 + 8],
                               ps[:, :, b0:b0 + 8]
                               .rearrange("p d b -> p b d"))

            def drain_one(t, h):
                copy_one(t, h)
                eng = nc.sync if t == 0 else nc.scalar
                eng.dma_start(out_d.ap()[:, t, 8 * h:8 * h + 8],
                              stage[:, t, 8 * h:8 * h + 8])

            # kt3 processes its h1 samples first so that only h0's final
            # groups sit in the end-of-kernel drain tail
            for kt in range(NKT):
                order = (list(range(8, 16)) + list(range(8))
                         if kt == NKT - 1 else list(range(B)))
                for u in range(B):
                    s = order[u]
                    g = g_psum.tile([128, 4, 256], F32, tag="g",
                                    name=f"g{kt}_{s}")
                    n0 = s * P
                    for m in range(4):
                        for xv, pv in ((0, 0), (1, 0), (0, 1)):
                            for q in range(2):
                                nc.tensor.matmul(
                                    g[:, m, 0:P],
                                    lhsT=pt[:, pv, kt, q, :, m, :],
                                    rhs=xt[:, xv, 2 * q:2 * q + 2, n0:n0 + P],
                                    start=(m % 2 == 0 and xv == 0 and pv == 0
                                           and q == 0),
                                    stop=(m % 2 == 1 and pv == 1 and q == 1),
                                    perf_mode=DR,
                                    skip_group_check=True,
                                )
                    # ACT: psum f32 -> sbuf bf16; 2 samples per vector group
                    fast = (kt == NKT - 1 and u == B - 1)
                    gw = 2
                    if u % 2 == 0:
                        sbt = sb_pool.tile([128, 4, gw, P], BF16,
                                           tag=f"sb{gw}", name=f"sb{kt}_{s}")
                    nc.scalar.copy(sbt[:, :, u % 2, :], g[:, :, 0:P])
                    if u % 2 == 1:
                        # DVE computes (RR, II) and the Re combine locally;
                        # Pool computes the (RI, IR) pair in one negative-
                        # stride mul so only the Im combine crosses engines.
                        pr = prod_pool.tile([128, 4, gw, P], BF16,
                                            tag=f"pr{gw}", name=f"pr{kt}_{s}")
                        nc.vector.tensor_mul(pr[:, 0:2], sbt[:, 0:2],
                                             sbt[:, 2:4])
                        nc.gpsimd.tensor_mul(pr[:, 2:4], sbt[:, 0:2],
                                             sbt[:, 3:1:-1])
                        # (cm free-dim padded to 256 so the AP optimizer
                        # cannot merge dims — the reduce window must be 196)
                        cm = c_pool.tile([128, 2, gw, 256], BF16,
                                         tag=f"cm{gw}", name=f"cm{kt}_{s}")
                        nc.vector.tensor_sub(cm[:, 0, :, 0:P], pr[:, 0],
                                             pr[:, 1])
                        flush_reduce()
                        nc.vector.tensor_add(cm[:, 1, :, 0:P], pr[:, 2],
                                             pr[:, 3])
                        pending.append((kt, s, gw, cm))
                        if fast:
                            flush_reduce()
                    # stage-3 half-batches: a half's slot sits >= 5 groups
                    # (~10us of vector-pipe drain) after its last sample;
                    # kt0's slots additionally wait out the cc DMA arrival
                    if kt >= 1 and u == (4 if kt == 1 else 2):
                        stage3(kt - 1, 0)
                    if kt >= 1 and u == 10:
                        stage3(kt - 1, 1)
            stage3(NKT - 1, 1)
            drain_one(0, 1)
            drain_one(1, 1)
            stage3(NKT - 1, 0, final=True)

    nc.compile()
    return nc


def make_constants(S1, S2):
    """Host-side constant prep from the sketch matrices (per-core slices)."""
    S1 = np.asarray(S1, np.float64)
    S2 = np.asarray(S2, np.float64)
    Phi = np.zeros((4, C, KPAD), np.float32)
    for i, S in enumerate((S1, S2)):
        F = np.fft.fft(S, axis=1)[:, :KH]
        Phi[2 * i, :, :KH] = F.real.astype(np.float32)
        Phi[2 * i + 1, :, :KH] = F.imag.astype(np.float32)

    # fp8 hi/lo split of Phi (lo rides e4m3 subnormals)
    p8 = Phi.astype(E4)
    plo = (Phi - p8.astype(np.float32)).astype(E4)

    # phi layout [128, v, kt, q, i, m, f]; v = (p8, plo)
    phis = []
    stack = np.stack([p8, plo], 0)  # [v, m, c, k]
    arr = stack.reshape(2, 4, 2, 2, 128, NCORES, NKT, 128)  # v m q i p j kt f
    for j in range(NCORES):
        a = arr[:, :, :, :, :, j]  # [v, m, q, i, p, kt, f]
        a = np.transpose(a, (4, 0, 5, 2, 3, 1, 6))  # p v kt q i m f
        phis.append(np.ascontiguousarray(a))

    # inverse half-DFT tables, d/D-d folded
    k = np.arange(KPAD, dtype=np.float64)
    wk = np.where((k == 0) | (k == D // 2), 1.0, 2.0) / D
    wk[KH:] = 0.0
    ang = 2.0 * np.pi * np.outer(k, np.arange(DH, dtype=np.float64)) / D
    Cst = np.zeros((2, KPAD, NDT * 128), np.float32)
    Cst[0, :, :DH] = wk[:, None] * np.cos(ang)
    Cst[1, :, :DH] = -wk[:, None] * np.sin(ang)
    Cst = Cst.astype(ml_dtypes.bfloat16)
    ccs = [np.ascontiguousarray(
        Cst.reshape(2, NCORES, NKT, 128, NDT * 128)[:, j]) for j in range(NCORES)]
    return phis, ccs


def prep_x(x):
    """[B, C, H, W] -> [128, 2, cc, BP] fp8 (x8, xlo)."""
    xr = np.ascontiguousarray(
        np.asarray(x, np.float32).reshape(B, C, P).transpose(1, 0, 2)
    ).reshape(C, BP)
    x8 = xr.astype(E4)
    xlo = (xr - x8.astype(np.float32)).astype(E4)
    out = np.empty((128, 2, NCC, BP), E4)
    for v, t in enumerate((x8, xlo)):
        out[:, v] = t.reshape(NCC, 128, BP).transpose(1, 0, 2)
    return out


def unshard(parts):
    """Sum per-core [128, 2, NDT, B] bf16 partials -> [B, D] f32."""
    acc = np.zeros((2, NDT * 128, B), np.float32)
    for r in parts:
        a = np.asarray(r, np.float32)  # [128, 2, B, NDT]
        acc += a.transpose(1, 3, 0, 2).reshape(2, NDT * 128, B)
    A, Bm = acc[0], acc[1]
    out = np.zeros((D, B), np.float32)
    out[:KH] = A[:KH] + Bm[:KH]
    out[KH:] = (A[1:4000] - Bm[1:4000])[::-1]
    return np.ascontiguousarray(out.T)


_CACHE = {}


def kernel(x, S1, S2):
    x = np.asarray(x)
    if "k" not in _CACHE:
        phis, ccs = make_constants(np.asarray(S1), np.asarray(S2))
        _CACHE["k"] = (build_nc(), phis, ccs)
    nc, phis, ccs = _CACHE["k"]

    xp = prep_x(x)
    in_maps = [{"x": xp, "phi": phis[j], "cc": ccs[j]} for j in range(NCORES)]
    res = run_bass_kernel_spmd(nc, in_maps, list(range(NCORES)))
    return unshard([r["out"] for r in res.results]).astype(x.dtype)
